# revision 25
# baseline (speedup 1.0000x reference)
"""Trainium2 kernel for nn_LocalMatchingLoss.

Strategy (data-parallel over batch, 3 samples per core on 8 cores):
  host:   fp32 normalize of geom/text features, gather active text rows
          (conf > 0.5, padded to K_PAD), stage one partition-major
          interleaved tensor per sample chunk: row [p, k] holds the z
          chunk (256 cols) then the active-t chunk (K_PAD cols), so each
          DMA piece is large contiguous runs.
  device: per sample, S[r, c] = sum_d zhat[r, d] * that_act[c, d] via 16
          accumulating 128-contraction matmuls per 128-row output half
          (both halves' PSUM groups open simultaneously so the post-DMA
          tail is tiny), DVE copy PSUM -> SBUF, DMA out [256, K_PAD].
          A short zero-matmul warm-up burst during the first DMA flips
          the PE HAM throttle to full clock before real work arrives.
  host:   Hungarian assignment on the 1 - S cost (scipy; fallback:
          pure-numpy JV), weighted mean loss. Mirrors the reference math.
"""

import os
import sys
import numpy as np

for _p in ("/opt/trn_rl_repo", "/root/.axon_site/_ro/trn_rl_repo"):
    if os.path.isdir(_p) and _p not in sys.path:
        sys.path.insert(0, _p)

from concourse import bacc, bass, mybir, tile
from concourse.bass_utils import run_bass_kernel_spmd

B, N_D, D = 24, 256, 2048
N_CORES = 8
PER_CORE = B // N_CORES          # 3
KCH = D // 128                   # 16 contraction chunks
CONF_THRESH = 0.5
EPS = 1e-12

# "f32r" (full-rate fp32 PE mode), "f32" (4 cyc/row), or "bf16"
MM_DTYPE = os.environ.get("KERNEL_MM_DTYPE", "bf16")

# Graduated DMA piece sizes (k-chunk ranges per sample): small leading
# pieces let the PE start ~4us earlier; a small final piece shrinks the
# post-last-byte tail.
PIECES = {
    0: ((0, 1), (1, 2), (2, 4), (4, 8), (8, 12), (12, 16)),
    1: ((0, 4), (4, 8), (8, 12), (12, 16)),
    2: ((0, 4), (4, 8), (8, 12), (12, 15), (15, 16)),
}

# Populated with the BassKernelResults of the last run when tracing is on
# (KERNEL_TRACE=1 / BASS_TRACE=1); used by test.py for HW timing.
LAST_RESULTS = None

_PROGRAM_CACHE = {}


def _build_program(mm_dtype, k_pad):
    """Trace the per-core Bass program (identical on all 8 cores)."""
    if mm_dtype == "bf16":
        in_dt = mybir.dt.bfloat16
    elif mm_dtype == "f32r":
        in_dt = mybir.dt.float32r
    else:
        in_dt = mybir.dt.float32
    cols = N_D + k_pad

    # Bacc (not bare Bass): its finalize() runs the wait-splitting passes
    # (move_matmul_waits_to_ldweights / generate_event_semaphores) that the
    # TRN2 1-wait-per-instruction constraint requires.
    nc = bacc.Bacc(None, target_bir_lowering=False)
    zt = nc.dram_tensor("zt", [PER_CORE, 128, KCH, cols], in_dt, kind="ExternalInput")
    # fp16 output: |S| <= ~0.2, so fp16's absolute error (~1.5e-5 at that
    # magnitude) is below the bf16-input noise floor; halves output bytes.
    s_out = nc.dram_tensor(
        "s", [PER_CORE, N_D, k_pad], mybir.dt.float16, kind="ExternalOutput"
    )

    with tile.TileContext(nc) as tc:
        with (
            tc.tile_pool(name="inp", bufs=1) as inp,
            tc.tile_pool(name="ps", bufs=3, space=bass.MemorySpace.PSUM) as psp,
            tc.tile_pool(name="wps", bufs=1, space=bass.MemorySpace.PSUM) as wpsp,
            tc.tile_pool(name="outp", bufs=3) as outp,
        ):
            # PE warm-up: ~2.5us of zero matmuls while the first DMA lands,
            # so the HAM clock gate is at 8/8 when real matmuls start.
            warm_dt = mybir.dt.float32 if mm_dtype == "f32r" else in_dt
            warm = inp.tile([128, 256], warm_dt, tag="warm")
            nc.vector.memset(warm[:], 0.0)
            wps = wpsp.tile([128, 256], mybir.dt.float32, tag="warmps")
            n_warm = 7 if warm_dt == mybir.dt.float32 else 20
            for i in range(n_warm):
                nc.tensor.matmul(
                    wps[:], warm[:, 0:128], warm[:],
                    start=(i == 0), stop=(i == n_warm - 1),
                )

            ring = [nc.sync, nc.scalar]
            n_dma = 0
            for smp in range(PER_CORE):
                in_t = inp.tile([128, KCH, cols], in_dt, tag=f"in{smp}")
                for k0, k1 in PIECES[smp]:
                    # alternate input pieces across the two HWDGE rings
                    ring[n_dma % 2].dma_start(
                        out=in_t[:, k0:k1, :], in_=zt[smp, :, k0:k1, :]
                    )
                    n_dma += 1
                pss = [
                    psp.tile([128, k_pad], mybir.dt.float32, tag="ps0", name=f"ps0_{smp}"),
                    psp.tile([128, k_pad], mybir.dt.float32, tag="ps1", name=f"ps1_{smp}"),
                ]
                o_t = outp.tile([128, 2, k_pad], mybir.dt.float16, tag="o")
                for k in range(KCH):
                    for h in range(2):
                        nc.tensor.matmul(
                            pss[h][:],
                            in_t[:, k, h * 128 : (h + 1) * 128],
                            in_t[:, k, N_D : N_D + k_pad],
                            start=(k == 0),
                            stop=(k == KCH - 1),
                        )
                nc.vector.tensor_copy(o_t[:, 0, :], pss[0][:])
                nc.scalar.copy(o_t[:, 1, :], pss[1][:])
                ring[n_dma % 2].dma_start(
                    out=s_out[smp].rearrange("(h p) c -> p h c", h=2), in_=o_t[:]
                )
                n_dma += 1
    nc.finalize()
    return nc


def _get_program(mm_dtype, k_pad):
    key = (mm_dtype, k_pad)
    if key not in _PROGRAM_CACHE:
        _PROGRAM_CACHE[key] = _build_program(mm_dtype, k_pad)
    return _PROGRAM_CACHE[key]


def _lsa(cost):
    """Jonker-Volgenant shortest-augmenting-path linear sum assignment.
    cost: (n, m) numpy array with n <= m. Returns (row_ind, col_ind)."""
    n, m = cost.shape
    INF = np.inf
    u = np.zeros(n + 1)
    v = np.zeros(m + 1)
    p = np.zeros(m + 1, dtype=np.int64)
    way = np.zeros(m + 1, dtype=np.int64)
    for i in range(1, n + 1):
        p[0] = i
        j0 = 0
        minv = np.full(m + 1, INF)
        used = np.zeros(m + 1, dtype=bool)
        while True:
            used[j0] = True
            i0 = p[j0]
            cand = cost[i0 - 1] - u[i0] - v[1:]
            upd = (~used[1:]) & (cand < minv[1:])
            minv[1:][upd] = cand[upd]
            way[1:][upd] = j0
            masked = np.where(used[1:], INF, minv[1:])
            j1 = int(np.argmin(masked)) + 1
            delta = masked[j1 - 1]
            uj = np.where(used)[0]
            u[p[uj]] += delta
            v[uj] -= delta
            minv[1:][~used[1:]] -= delta
            j0 = j1
            if p[j0] == 0:
                break
        while j0 != 0:
            j1 = way[j0]
            p[j0] = p[j1]
            j0 = j1
    rows = []
    cols = []
    for j in range(1, m + 1):
        if p[j] != 0:
            rows.append(p[j] - 1)
            cols.append(j - 1)
    return np.asarray(rows, dtype=np.int64), np.asarray(cols, dtype=np.int64)


def _solve_assignment(costT):
    """costT: (K, N_D) with K <= N_D. Returns (t_idx, g_idx)."""
    try:
        from scipy.optimize import linear_sum_assignment
    except ImportError:
        return _lsa(costT)
    r, c = linear_sum_assignment(costT)
    return np.asarray(r, dtype=np.int64), np.asarray(c, dtype=np.int64)


def _normalize(x):
    n = np.maximum(np.linalg.norm(x, axis=-1, keepdims=True), np.float32(EPS))
    return (x / n).astype(np.float32)


def kernel(geom_features, text_features, text_confidence):
    global LAST_RESULTS
    geom = np.asarray(geom_features, dtype=np.float32)
    text = np.asarray(text_features, dtype=np.float32)
    conf = np.asarray(text_confidence, dtype=np.float32)

    acts = [np.where(conf[b] > CONF_THRESH)[0] for b in range(B)]
    k_max = max(a.size for a in acts)
    if k_max == 0:
        return np.float32(0.0)
    k_pad = min(N_D, ((max(k_max, 32) + 7) // 8) * 8)

    np_dt = np.float32
    if MM_DTYPE == "bf16":
        import ml_dtypes

        np_dt = ml_dtypes.bfloat16

    zn = _normalize(geom)   # (B, N_D, D)
    tn = _normalize(text)
    # staging layout: zt[b, p, k, 0:256]   = zhat[b, c, k*128+p]
    #                 zt[b, p, k, 256:] = that_act[b, c_local, k*128+p], zero pad
    zt = np.zeros((B, 128, KCH, N_D + k_pad), dtype=np_dt)
    zt[:, :, :, :N_D] = zn.reshape(B, N_D, KCH, 128).transpose(0, 3, 2, 1)
    for b in range(B):
        a = acts[b]
        if a.size:
            zt[b, :, :, N_D : N_D + a.size] = (
                tn[b, a].reshape(a.size, KCH, 128).transpose(2, 1, 0)
            )

    nc = _get_program(MM_DTYPE, k_pad)
    in_maps = [
        {"zt": zt[i * PER_CORE : (i + 1) * PER_CORE]} for i in range(N_CORES)
    ]
    trace = os.environ.get("KERNEL_TRACE", "0") == "1"
    res = run_bass_kernel_spmd(nc, in_maps, core_ids=list(range(N_CORES)), trace=trace)
    if trace:
        LAST_RESULTS = res
    S = np.concatenate([r["s"] for r in res.results], axis=0).astype(
        np.float32
    )  # (B, N_D, k_pad)

    total = np.float32(0.0)
    valid = 0
    for b in range(B):
        a = acts[b]
        if a.size == 0:
            continue
        cost = (np.float32(1.0) - S[b][:, : a.size]).astype(np.float32)  # [N_D, K]
        t_idx, g_idx = _solve_assignment(cost.T)
        pair_cost = cost[g_idx, t_idx]
        w = conf[b, a][t_idx]
        total = total + np.float32(np.sum((w * pair_cost).astype(np.float32))) / np.float32(
            g_idx.shape[0]
        )
        valid += 1
    if valid == 0:
        return np.float32(0.0)
    return np.float32(total / np.float32(valid))


# revision 27
# speedup vs baseline: 1.0078x; 1.0078x over previous
"""Trainium2 kernel for nn_LocalMatchingLoss.

Strategy (data-parallel over batch, 3 samples per core on 8 cores):
  host:   fp32 normalize of geom/text features, gather active text rows
          (conf > 0.5, padded to K_PAD), stage one partition-major
          interleaved tensor per sample chunk: row [p, k] holds the z
          chunk (256 cols) then the active-t chunk (K_PAD cols), so each
          DMA piece is large contiguous runs.
  device: per sample, S[r, c] = sum_d zhat[r, d] * that_act[c, d] via 16
          accumulating 128-contraction matmuls per 128-row output half
          (both halves' PSUM groups open simultaneously so the post-DMA
          tail is tiny), DVE copy PSUM -> SBUF, DMA out [256, K_PAD].
          A short zero-matmul warm-up burst during the first DMA flips
          the PE HAM throttle to full clock before real work arrives.
  host:   Hungarian assignment on the 1 - S cost (scipy; fallback:
          pure-numpy JV), weighted mean loss. Mirrors the reference math.
"""

import os
import sys
import numpy as np

for _p in ("/opt/trn_rl_repo", "/root/.axon_site/_ro/trn_rl_repo"):
    if os.path.isdir(_p) and _p not in sys.path:
        sys.path.insert(0, _p)

from concourse import bacc, bass, mybir, tile
from concourse.bass_utils import run_bass_kernel_spmd

B, N_D, D = 24, 256, 2048
N_CORES = 8
PER_CORE = B // N_CORES          # 3
KCH = D // 128                   # 16 contraction chunks
CONF_THRESH = 0.5
EPS = 1e-12

# "f32r" (full-rate fp32 PE mode), "f32" (4 cyc/row), or "bf16"
MM_DTYPE = os.environ.get("KERNEL_MM_DTYPE", "bf16")

# Graduated DMA piece sizes (k-chunk ranges per sample): small leading
# pieces let the PE start ~4us earlier; a small final piece shrinks the
# post-last-byte tail.
PIECES = {
    0: ((0, 1), (1, 2), (2, 4), (4, 8), (8, 12), (12, 16)),
    1: ((0, 4), (4, 8), (8, 12), (12, 16)),
    2: ((0, 4), (4, 8), (8, 12), (12, 15), (15, 16)),
}

# Populated with the BassKernelResults of the last run when tracing is on
# (KERNEL_TRACE=1 / BASS_TRACE=1); used by test.py for HW timing.
LAST_RESULTS = None

_PROGRAM_CACHE = {}


def _build_program(mm_dtype, k_pad):
    """Trace the per-core Bass program (identical on all 8 cores)."""
    if mm_dtype == "bf16":
        in_dt = mybir.dt.bfloat16
    elif mm_dtype == "f32r":
        in_dt = mybir.dt.float32r
    else:
        in_dt = mybir.dt.float32
    cols = N_D + k_pad

    # Bacc (not bare Bass): its finalize() runs the wait-splitting passes
    # (move_matmul_waits_to_ldweights / generate_event_semaphores) that the
    # TRN2 1-wait-per-instruction constraint requires.
    nc = bacc.Bacc(None, target_bir_lowering=False)
    zt = nc.dram_tensor("zt", [PER_CORE, 128, KCH, cols], in_dt, kind="ExternalInput")
    # fp16 output: |S| <= ~0.2, so fp16's absolute error (~1.5e-5 at that
    # magnitude) is below the bf16-input noise floor; halves output bytes.
    s_out = nc.dram_tensor(
        "s", [PER_CORE, N_D, k_pad], mybir.dt.float16, kind="ExternalOutput"
    )

    with tile.TileContext(nc) as tc:
        with (
            tc.tile_pool(name="inp", bufs=1) as inp,
            tc.tile_pool(name="ps", bufs=3, space=bass.MemorySpace.PSUM) as psp,
            tc.tile_pool(name="wps", bufs=1, space=bass.MemorySpace.PSUM) as wpsp,
            tc.tile_pool(name="outp", bufs=3) as outp,
        ):
            # PE warm-up: ~2.5us of zero matmuls while the first DMA lands,
            # so the HAM clock gate is at 8/8 when real matmuls start.
            warm_dt = mybir.dt.float32 if mm_dtype == "f32r" else in_dt
            warm = inp.tile([128, 256], warm_dt, tag="warm")
            nc.vector.memset(warm[:], 0.0)
            wps = wpsp.tile([128, 256], mybir.dt.float32, tag="warmps")
            n_warm = 5 if warm_dt == mybir.dt.float32 else 14
            for i in range(n_warm):
                nc.tensor.matmul(
                    wps[:], warm[:, 0:128], warm[:],
                    start=(i == 0), stop=(i == n_warm - 1),
                )

            ring = [nc.sync, nc.scalar]
            n_dma = 0
            for smp in range(PER_CORE):
                in_t = inp.tile([128, KCH, cols], in_dt, tag=f"in{smp}")
                for k0, k1 in PIECES[smp]:
                    # alternate input pieces across the two HWDGE rings
                    ring[n_dma % 2].dma_start(
                        out=in_t[:, k0:k1, :], in_=zt[smp, :, k0:k1, :]
                    )
                    n_dma += 1
                pss = [
                    psp.tile([128, k_pad], mybir.dt.float32, tag="ps0", name=f"ps0_{smp}"),
                    psp.tile([128, k_pad], mybir.dt.float32, tag="ps1", name=f"ps1_{smp}"),
                ]
                o_t = outp.tile([128, 2, k_pad], mybir.dt.float16, tag="o")
                for k in range(KCH):
                    for h in range(2):
                        nc.tensor.matmul(
                            pss[h][:],
                            in_t[:, k, h * 128 : (h + 1) * 128],
                            in_t[:, k, N_D : N_D + k_pad],
                            start=(k == 0),
                            stop=(k == KCH - 1),
                        )
                for h in range(2):
                    nc.vector.tensor_copy(o_t[:, h, :], pss[h][:])
                ring[n_dma % 2].dma_start(
                    out=s_out[smp].rearrange("(h p) c -> p h c", h=2), in_=o_t[:]
                )
                n_dma += 1
    nc.finalize()
    return nc


def _get_program(mm_dtype, k_pad):
    key = (mm_dtype, k_pad)
    if key not in _PROGRAM_CACHE:
        _PROGRAM_CACHE[key] = _build_program(mm_dtype, k_pad)
    return _PROGRAM_CACHE[key]


def _lsa(cost):
    """Jonker-Volgenant shortest-augmenting-path linear sum assignment.
    cost: (n, m) numpy array with n <= m. Returns (row_ind, col_ind)."""
    n, m = cost.shape
    INF = np.inf
    u = np.zeros(n + 1)
    v = np.zeros(m + 1)
    p = np.zeros(m + 1, dtype=np.int64)
    way = np.zeros(m + 1, dtype=np.int64)
    for i in range(1, n + 1):
        p[0] = i
        j0 = 0
        minv = np.full(m + 1, INF)
        used = np.zeros(m + 1, dtype=bool)
        while True:
            used[j0] = True
            i0 = p[j0]
            cand = cost[i0 - 1] - u[i0] - v[1:]
            upd = (~used[1:]) & (cand < minv[1:])
            minv[1:][upd] = cand[upd]
            way[1:][upd] = j0
            masked = np.where(used[1:], INF, minv[1:])
            j1 = int(np.argmin(masked)) + 1
            delta = masked[j1 - 1]
            uj = np.where(used)[0]
            u[p[uj]] += delta
            v[uj] -= delta
            minv[1:][~used[1:]] -= delta
            j0 = j1
            if p[j0] == 0:
                break
        while j0 != 0:
            j1 = way[j0]
            p[j0] = p[j1]
            j0 = j1
    rows = []
    cols = []
    for j in range(1, m + 1):
        if p[j] != 0:
            rows.append(p[j] - 1)
            cols.append(j - 1)
    return np.asarray(rows, dtype=np.int64), np.asarray(cols, dtype=np.int64)


def _solve_assignment(costT):
    """costT: (K, N_D) with K <= N_D. Returns (t_idx, g_idx)."""
    try:
        from scipy.optimize import linear_sum_assignment
    except ImportError:
        return _lsa(costT)
    r, c = linear_sum_assignment(costT)
    return np.asarray(r, dtype=np.int64), np.asarray(c, dtype=np.int64)


def _normalize(x):
    n = np.maximum(np.linalg.norm(x, axis=-1, keepdims=True), np.float32(EPS))
    return (x / n).astype(np.float32)


def kernel(geom_features, text_features, text_confidence):
    global LAST_RESULTS
    geom = np.asarray(geom_features, dtype=np.float32)
    text = np.asarray(text_features, dtype=np.float32)
    conf = np.asarray(text_confidence, dtype=np.float32)

    acts = [np.where(conf[b] > CONF_THRESH)[0] for b in range(B)]
    k_max = max(a.size for a in acts)
    if k_max == 0:
        return np.float32(0.0)
    k_pad = min(N_D, ((max(k_max, 32) + 7) // 8) * 8)

    np_dt = np.float32
    if MM_DTYPE == "bf16":
        import ml_dtypes

        np_dt = ml_dtypes.bfloat16

    zn = _normalize(geom)   # (B, N_D, D)
    tn = _normalize(text)
    # staging layout: zt[b, p, k, 0:256]   = zhat[b, c, k*128+p]
    #                 zt[b, p, k, 256:] = that_act[b, c_local, k*128+p], zero pad
    zt = np.zeros((B, 128, KCH, N_D + k_pad), dtype=np_dt)
    zt[:, :, :, :N_D] = zn.reshape(B, N_D, KCH, 128).transpose(0, 3, 2, 1)
    for b in range(B):
        a = acts[b]
        if a.size:
            zt[b, :, :, N_D : N_D + a.size] = (
                tn[b, a].reshape(a.size, KCH, 128).transpose(2, 1, 0)
            )

    nc = _get_program(MM_DTYPE, k_pad)
    in_maps = [
        {"zt": zt[i * PER_CORE : (i + 1) * PER_CORE]} for i in range(N_CORES)
    ]
    trace = os.environ.get("KERNEL_TRACE", "0") == "1"
    res = run_bass_kernel_spmd(nc, in_maps, core_ids=list(range(N_CORES)), trace=trace)
    if trace:
        LAST_RESULTS = res
    S = np.concatenate([r["s"] for r in res.results], axis=0).astype(
        np.float32
    )  # (B, N_D, k_pad)

    total = np.float32(0.0)
    valid = 0
    for b in range(B):
        a = acts[b]
        if a.size == 0:
            continue
        cost = (np.float32(1.0) - S[b][:, : a.size]).astype(np.float32)  # [N_D, K]
        t_idx, g_idx = _solve_assignment(cost.T)
        pair_cost = cost[g_idx, t_idx]
        w = conf[b, a][t_idx]
        total = total + np.float32(np.sum((w * pair_cost).astype(np.float32))) / np.float32(
            g_idx.shape[0]
        )
        valid += 1
    if valid == 0:
        return np.float32(0.0)
    return np.float32(total / np.float32(valid))


# revision 28
# speedup vs baseline: 1.0863x; 1.0779x over previous
"""Trainium2 kernel for nn_LocalMatchingLoss.

Strategy (data-parallel over batch, 3 samples per core on 8 cores):
  host:   fp32 normalize of geom/text features, gather active text rows
          (conf > 0.5, padded to K_PAD), stage one partition-major
          interleaved tensor per sample chunk: row [p, k] holds the z
          chunk (256 cols) then the active-t chunk (K_PAD cols), so each
          DMA piece is large contiguous runs.
  device: per sample, S[r, c] = sum_d zhat[r, d] * that_act[c, d] via 16
          accumulating 128-contraction matmuls per 128-row output half
          (both halves' PSUM groups open simultaneously so the post-DMA
          tail is tiny), DVE copy PSUM -> SBUF, DMA out [256, K_PAD].
          A short zero-matmul warm-up burst during the first DMA flips
          the PE HAM throttle to full clock before real work arrives.
  host:   Hungarian assignment on the 1 - S cost (scipy; fallback:
          pure-numpy JV), weighted mean loss. Mirrors the reference math.
"""

import os
import sys
import numpy as np

for _p in ("/opt/trn_rl_repo", "/root/.axon_site/_ro/trn_rl_repo"):
    if os.path.isdir(_p) and _p not in sys.path:
        sys.path.insert(0, _p)

from concourse import bacc, bass, mybir, tile
from concourse.bass_utils import run_bass_kernel_spmd

B, N_D, D = 24, 256, 2048
N_CORES = 8
PER_CORE = B // N_CORES          # 3
KCH = D // 128                   # 16 contraction chunks
CONF_THRESH = 0.5
EPS = 1e-12

# "f32r" (full-rate fp32 PE mode), "f32" (4 cyc/row), or "bf16"
MM_DTYPE = os.environ.get("KERNEL_MM_DTYPE", "bf16")

# Graduated DMA piece sizes (k-chunk ranges per sample): small leading
# pieces let the PE start ~4us earlier; a small final piece shrinks the
# post-last-byte tail.
PIECES = {
    0: ((0, 1), (1, 2), (2, 4), (4, 8), (8, 12), (12, 16)),
    1: ((0, 4), (4, 8), (8, 12), (12, 16)),
    2: ((0, 4), (4, 8), (8, 12), (12, 15), (15, 16)),
}

# Populated with the BassKernelResults of the last run when tracing is on
# (KERNEL_TRACE=1 / BASS_TRACE=1); used by test.py for HW timing.
LAST_RESULTS = None

_PROGRAM_CACHE = {}


def _build_program(mm_dtype, k_pad):
    """Trace the per-core Bass program (identical on all 8 cores)."""
    if mm_dtype == "bf16":
        in_dt = mybir.dt.bfloat16
    elif mm_dtype == "f32r":
        in_dt = mybir.dt.float32r
    else:
        in_dt = mybir.dt.float32
    cols = N_D + k_pad

    # Bacc (not bare Bass): its finalize() runs the wait-splitting passes
    # (move_matmul_waits_to_ldweights / generate_event_semaphores) that the
    # TRN2 1-wait-per-instruction constraint requires.
    nc = bacc.Bacc(None, target_bir_lowering=False)
    zt = nc.dram_tensor("zt", [PER_CORE, 128, KCH, cols], in_dt, kind="ExternalInput")
    # fp16 output: |S| <= ~0.2, so fp16's absolute error (~1.5e-5 at that
    # magnitude) is below the bf16-input noise floor; halves output bytes.
    s_out = nc.dram_tensor(
        "s", [PER_CORE, N_D, k_pad], mybir.dt.float16, kind="ExternalOutput"
    )

    with tile.TileContext(nc) as tc:
        with (
            tc.tile_pool(name="inp", bufs=1) as inp,
            tc.tile_pool(name="ps", bufs=3, space=bass.MemorySpace.PSUM) as psp,
            tc.tile_pool(name="wps", bufs=1, space=bass.MemorySpace.PSUM) as wpsp,
            tc.tile_pool(name="outp", bufs=3) as outp,
        ):
            # PE warm-up: ~2.5us of zero matmuls while the first DMA lands,
            # so the HAM clock gate is at 8/8 when real matmuls start.
            warm_dt = mybir.dt.float32 if mm_dtype == "f32r" else in_dt
            warm = inp.tile([128, 256], warm_dt, tag="warm")
            nc.vector.memset(warm[:], 0.0)
            wps = wpsp.tile([128, 256], mybir.dt.float32, tag="warmps")
            n_warm = 5 if warm_dt == mybir.dt.float32 else 20
            for i in range(n_warm):
                nc.tensor.matmul(
                    wps[:], warm[:, 0:128], warm[:],
                    start=(i == 0), stop=(i == n_warm - 1),
                )

            ring = [nc.sync, nc.scalar]
            n_dma = 0
            for smp in range(PER_CORE):
                in_t = inp.tile([128, KCH, cols], in_dt, tag=f"in{smp}")
                for k0, k1 in PIECES[smp]:
                    # alternate input pieces across the two HWDGE rings
                    ring[n_dma % 2].dma_start(
                        out=in_t[:, k0:k1, :], in_=zt[smp, :, k0:k1, :]
                    )
                    n_dma += 1
                pss = [
                    psp.tile([128, k_pad], mybir.dt.float32, tag="ps0", name=f"ps0_{smp}"),
                    psp.tile([128, k_pad], mybir.dt.float32, tag="ps1", name=f"ps1_{smp}"),
                ]
                o_t = outp.tile([128, 2, k_pad], mybir.dt.float16, tag="o")
                for k in range(KCH):
                    for h in range(2):
                        nc.tensor.matmul(
                            pss[h][:],
                            in_t[:, k, h * 128 : (h + 1) * 128],
                            in_t[:, k, N_D : N_D + k_pad],
                            start=(k == 0),
                            stop=(k == KCH - 1),
                        )
                for h in range(2):
                    nc.vector.tensor_copy(o_t[:, h, :], pss[h][:])
                ring[n_dma % 2].dma_start(
                    out=s_out[smp].rearrange("(h p) c -> p h c", h=2), in_=o_t[:]
                )
                n_dma += 1
    nc.finalize()
    return nc


def _get_program(mm_dtype, k_pad):
    key = (mm_dtype, k_pad)
    if key not in _PROGRAM_CACHE:
        _PROGRAM_CACHE[key] = _build_program(mm_dtype, k_pad)
    return _PROGRAM_CACHE[key]


def _lsa(cost):
    """Jonker-Volgenant shortest-augmenting-path linear sum assignment.
    cost: (n, m) numpy array with n <= m. Returns (row_ind, col_ind)."""
    n, m = cost.shape
    INF = np.inf
    u = np.zeros(n + 1)
    v = np.zeros(m + 1)
    p = np.zeros(m + 1, dtype=np.int64)
    way = np.zeros(m + 1, dtype=np.int64)
    for i in range(1, n + 1):
        p[0] = i
        j0 = 0
        minv = np.full(m + 1, INF)
        used = np.zeros(m + 1, dtype=bool)
        while True:
            used[j0] = True
            i0 = p[j0]
            cand = cost[i0 - 1] - u[i0] - v[1:]
            upd = (~used[1:]) & (cand < minv[1:])
            minv[1:][upd] = cand[upd]
            way[1:][upd] = j0
            masked = np.where(used[1:], INF, minv[1:])
            j1 = int(np.argmin(masked)) + 1
            delta = masked[j1 - 1]
            uj = np.where(used)[0]
            u[p[uj]] += delta
            v[uj] -= delta
            minv[1:][~used[1:]] -= delta
            j0 = j1
            if p[j0] == 0:
                break
        while j0 != 0:
            j1 = way[j0]
            p[j0] = p[j1]
            j0 = j1
    rows = []
    cols = []
    for j in range(1, m + 1):
        if p[j] != 0:
            rows.append(p[j] - 1)
            cols.append(j - 1)
    return np.asarray(rows, dtype=np.int64), np.asarray(cols, dtype=np.int64)


def _solve_assignment(costT):
    """costT: (K, N_D) with K <= N_D. Returns (t_idx, g_idx)."""
    try:
        from scipy.optimize import linear_sum_assignment
    except ImportError:
        return _lsa(costT)
    r, c = linear_sum_assignment(costT)
    return np.asarray(r, dtype=np.int64), np.asarray(c, dtype=np.int64)


def _normalize(x):
    n = np.maximum(np.linalg.norm(x, axis=-1, keepdims=True), np.float32(EPS))
    return (x / n).astype(np.float32)


def kernel(geom_features, text_features, text_confidence):
    global LAST_RESULTS
    geom = np.asarray(geom_features, dtype=np.float32)
    text = np.asarray(text_features, dtype=np.float32)
    conf = np.asarray(text_confidence, dtype=np.float32)

    acts = [np.where(conf[b] > CONF_THRESH)[0] for b in range(B)]
    k_max = max(a.size for a in acts)
    if k_max == 0:
        return np.float32(0.0)
    k_pad = min(N_D, ((max(k_max, 32) + 7) // 8) * 8)

    np_dt = np.float32
    if MM_DTYPE == "bf16":
        import ml_dtypes

        np_dt = ml_dtypes.bfloat16

    zn = _normalize(geom)   # (B, N_D, D)
    tn = _normalize(text)
    # staging layout: zt[b, p, k, 0:256]   = zhat[b, c, k*128+p]
    #                 zt[b, p, k, 256:] = that_act[b, c_local, k*128+p], zero pad
    zt = np.zeros((B, 128, KCH, N_D + k_pad), dtype=np_dt)
    zt[:, :, :, :N_D] = zn.reshape(B, N_D, KCH, 128).transpose(0, 3, 2, 1)
    for b in range(B):
        a = acts[b]
        if a.size:
            zt[b, :, :, N_D : N_D + a.size] = (
                tn[b, a].reshape(a.size, KCH, 128).transpose(2, 1, 0)
            )

    nc = _get_program(MM_DTYPE, k_pad)
    in_maps = [
        {"zt": zt[i * PER_CORE : (i + 1) * PER_CORE]} for i in range(N_CORES)
    ]
    trace = os.environ.get("KERNEL_TRACE", "0") == "1"
    res = run_bass_kernel_spmd(nc, in_maps, core_ids=list(range(N_CORES)), trace=trace)
    if trace:
        LAST_RESULTS = res
    S = np.concatenate([r["s"] for r in res.results], axis=0).astype(
        np.float32
    )  # (B, N_D, k_pad)

    total = np.float32(0.0)
    valid = 0
    for b in range(B):
        a = acts[b]
        if a.size == 0:
            continue
        cost = (np.float32(1.0) - S[b][:, : a.size]).astype(np.float32)  # [N_D, K]
        t_idx, g_idx = _solve_assignment(cost.T)
        pair_cost = cost[g_idx, t_idx]
        w = conf[b, a][t_idx]
        total = total + np.float32(np.sum((w * pair_cost).astype(np.float32))) / np.float32(
            g_idx.shape[0]
        )
        valid += 1
    if valid == 0:
        return np.float32(0.0)
    return np.float32(total / np.float32(valid))


# revision 29
# speedup vs baseline: 1.1123x; 1.0240x over previous
"""Trainium2 kernel for nn_LocalMatchingLoss.

Strategy (data-parallel over batch, 3 samples per core on 8 cores):
  host:   fp32 normalize of geom/text features, gather active text rows
          (conf > 0.5, padded to K_PAD), stage one partition-major
          interleaved tensor per sample chunk: row [p, k] holds the z
          chunk (256 cols) then the active-t chunk (K_PAD cols), so each
          DMA piece is large contiguous runs.
  device: per sample, S[r, c] = sum_d zhat[r, d] * that_act[c, d] via 16
          accumulating 128-contraction matmuls per 128-row output half
          (both halves' PSUM groups open simultaneously so the post-DMA
          tail is tiny), DVE copy PSUM -> SBUF, DMA out [256, K_PAD].
          A short zero-matmul warm-up burst during the first DMA flips
          the PE HAM throttle to full clock before real work arrives.
  host:   Hungarian assignment on the 1 - S cost (scipy; fallback:
          pure-numpy JV), weighted mean loss. Mirrors the reference math.
"""

import os
import sys
import numpy as np

for _p in ("/opt/trn_rl_repo", "/root/.axon_site/_ro/trn_rl_repo"):
    if os.path.isdir(_p) and _p not in sys.path:
        sys.path.insert(0, _p)

from concourse import bacc, bass, mybir, tile
from concourse.bass_utils import run_bass_kernel_spmd

B, N_D, D = 24, 256, 2048
N_CORES = 8
PER_CORE = B // N_CORES          # 3
KCH = D // 128                   # 16 contraction chunks
CONF_THRESH = 0.5
EPS = 1e-12

# "f32r" (full-rate fp32 PE mode), "f32" (4 cyc/row), or "bf16"
MM_DTYPE = os.environ.get("KERNEL_MM_DTYPE", "bf16")

# Graduated DMA piece sizes (k-chunk ranges per sample): small leading
# pieces let the PE start ~4us earlier; a small final piece shrinks the
# post-last-byte tail.
PIECES = {
    0: ((0, 1), (1, 2), (2, 4), (4, 8), (8, 12), (12, 16)),
    1: ((0, 4), (4, 8), (8, 12), (12, 16)),
    2: ((0, 4), (4, 8), (8, 12), (12, 15), (15, 16)),
}

# Populated with the BassKernelResults of the last run when tracing is on
# (KERNEL_TRACE=1 / BASS_TRACE=1); used by test.py for HW timing.
LAST_RESULTS = None

_PROGRAM_CACHE = {}


def _build_program(mm_dtype, k_pad):
    """Trace the per-core Bass program (identical on all 8 cores)."""
    if mm_dtype == "bf16":
        in_dt = mybir.dt.bfloat16
    elif mm_dtype == "f32r":
        in_dt = mybir.dt.float32r
    else:
        in_dt = mybir.dt.float32
    cols = N_D + k_pad

    # Bacc (not bare Bass): its finalize() runs the wait-splitting passes
    # (move_matmul_waits_to_ldweights / generate_event_semaphores) that the
    # TRN2 1-wait-per-instruction constraint requires.
    nc = bacc.Bacc(None, target_bir_lowering=False)
    zt = nc.dram_tensor("zt", [PER_CORE, 128, KCH, cols], in_dt, kind="ExternalInput")
    # fp16 output: |S| <= ~0.2, so fp16's absolute error (~1.5e-5 at that
    # magnitude) is below the bf16-input noise floor; halves output bytes.
    s_out = nc.dram_tensor(
        "s", [PER_CORE, N_D, k_pad], mybir.dt.float16, kind="ExternalOutput"
    )

    with tile.TileContext(nc) as tc:
        with (
            tc.tile_pool(name="inp", bufs=1) as inp,
            tc.tile_pool(name="ps", bufs=3, space=bass.MemorySpace.PSUM) as psp,
            tc.tile_pool(name="wps", bufs=1, space=bass.MemorySpace.PSUM) as wpsp,
            tc.tile_pool(name="outp", bufs=3) as outp,
        ):
            # PE warm-up: ~2.5us of zero matmuls while the first DMA lands,
            # so the HAM clock gate is at 8/8 when real matmuls start.
            warm_dt = mybir.dt.float32 if mm_dtype == "f32r" else in_dt
            warm = inp.tile([128, 256], warm_dt, tag="warm")
            nc.vector.memset(warm[:], 0.0)
            wps = wpsp.tile([128, 256], mybir.dt.float32, tag="warmps")
            n_warm = 5 if warm_dt == mybir.dt.float32 else 14
            for i in range(n_warm):
                nc.tensor.matmul(
                    wps[:], warm[:, 0:128], warm[:],
                    start=(i == 0), stop=(i == n_warm - 1),
                )

            ring = [nc.sync, nc.scalar]
            n_dma = 0
            for smp in range(PER_CORE):
                in_t = inp.tile([128, KCH, cols], in_dt, tag=f"in{smp}")
                for k0, k1 in PIECES[smp]:
                    # alternate input pieces across the two HWDGE rings
                    ring[n_dma % 2].dma_start(
                        out=in_t[:, k0:k1, :], in_=zt[smp, :, k0:k1, :]
                    )
                    n_dma += 1
                pss = [
                    psp.tile([128, k_pad], mybir.dt.float32, tag="ps0", name=f"ps0_{smp}"),
                    psp.tile([128, k_pad], mybir.dt.float32, tag="ps1", name=f"ps1_{smp}"),
                ]
                o_t = outp.tile([128, 2, k_pad], mybir.dt.float16, tag="o")
                for k in range(KCH):
                    for h in range(2):
                        nc.tensor.matmul(
                            pss[h][:],
                            in_t[:, k, h * 128 : (h + 1) * 128],
                            in_t[:, k, N_D : N_D + k_pad],
                            start=(k == 0),
                            stop=(k == KCH - 1),
                        )
                for h in range(2):
                    nc.vector.tensor_copy(o_t[:, h, :], pss[h][:])
                ring[n_dma % 2].dma_start(
                    out=s_out[smp].rearrange("(h p) c -> p h c", h=2), in_=o_t[:]
                )
                n_dma += 1
    nc.finalize()
    return nc


def _get_program(mm_dtype, k_pad):
    key = (mm_dtype, k_pad)
    if key not in _PROGRAM_CACHE:
        _PROGRAM_CACHE[key] = _build_program(mm_dtype, k_pad)
    return _PROGRAM_CACHE[key]


def _lsa(cost):
    """Jonker-Volgenant shortest-augmenting-path linear sum assignment.
    cost: (n, m) numpy array with n <= m. Returns (row_ind, col_ind)."""
    n, m = cost.shape
    INF = np.inf
    u = np.zeros(n + 1)
    v = np.zeros(m + 1)
    p = np.zeros(m + 1, dtype=np.int64)
    way = np.zeros(m + 1, dtype=np.int64)
    for i in range(1, n + 1):
        p[0] = i
        j0 = 0
        minv = np.full(m + 1, INF)
        used = np.zeros(m + 1, dtype=bool)
        while True:
            used[j0] = True
            i0 = p[j0]
            cand = cost[i0 - 1] - u[i0] - v[1:]
            upd = (~used[1:]) & (cand < minv[1:])
            minv[1:][upd] = cand[upd]
            way[1:][upd] = j0
            masked = np.where(used[1:], INF, minv[1:])
            j1 = int(np.argmin(masked)) + 1
            delta = masked[j1 - 1]
            uj = np.where(used)[0]
            u[p[uj]] += delta
            v[uj] -= delta
            minv[1:][~used[1:]] -= delta
            j0 = j1
            if p[j0] == 0:
                break
        while j0 != 0:
            j1 = way[j0]
            p[j0] = p[j1]
            j0 = j1
    rows = []
    cols = []
    for j in range(1, m + 1):
        if p[j] != 0:
            rows.append(p[j] - 1)
            cols.append(j - 1)
    return np.asarray(rows, dtype=np.int64), np.asarray(cols, dtype=np.int64)


def _solve_assignment(costT):
    """costT: (K, N_D) with K <= N_D. Returns (t_idx, g_idx)."""
    try:
        from scipy.optimize import linear_sum_assignment
    except ImportError:
        return _lsa(costT)
    r, c = linear_sum_assignment(costT)
    return np.asarray(r, dtype=np.int64), np.asarray(c, dtype=np.int64)


def _normalize(x):
    n = np.maximum(np.linalg.norm(x, axis=-1, keepdims=True), np.float32(EPS))
    return (x / n).astype(np.float32)


def kernel(geom_features, text_features, text_confidence):
    global LAST_RESULTS
    geom = np.asarray(geom_features, dtype=np.float32)
    text = np.asarray(text_features, dtype=np.float32)
    conf = np.asarray(text_confidence, dtype=np.float32)

    acts = [np.where(conf[b] > CONF_THRESH)[0] for b in range(B)]
    k_max = max(a.size for a in acts)
    if k_max == 0:
        return np.float32(0.0)
    k_pad = min(N_D, ((max(k_max, 32) + 7) // 8) * 8)

    np_dt = np.float32
    if MM_DTYPE == "bf16":
        import ml_dtypes

        np_dt = ml_dtypes.bfloat16

    zn = _normalize(geom)   # (B, N_D, D)
    tn = _normalize(text)
    # staging layout: zt[b, p, k, 0:256]   = zhat[b, c, k*128+p]
    #                 zt[b, p, k, 256:] = that_act[b, c_local, k*128+p], zero pad
    zt = np.zeros((B, 128, KCH, N_D + k_pad), dtype=np_dt)
    zt[:, :, :, :N_D] = zn.reshape(B, N_D, KCH, 128).transpose(0, 3, 2, 1)
    for b in range(B):
        a = acts[b]
        if a.size:
            zt[b, :, :, N_D : N_D + a.size] = (
                tn[b, a].reshape(a.size, KCH, 128).transpose(2, 1, 0)
            )

    nc = _get_program(MM_DTYPE, k_pad)
    in_maps = [
        {"zt": zt[i * PER_CORE : (i + 1) * PER_CORE]} for i in range(N_CORES)
    ]
    trace = os.environ.get("KERNEL_TRACE", "0") == "1"
    res = run_bass_kernel_spmd(nc, in_maps, core_ids=list(range(N_CORES)), trace=trace)
    if trace:
        LAST_RESULTS = res
    S = np.concatenate([r["s"] for r in res.results], axis=0).astype(
        np.float32
    )  # (B, N_D, k_pad)

    total = np.float32(0.0)
    valid = 0
    for b in range(B):
        a = acts[b]
        if a.size == 0:
            continue
        cost = (np.float32(1.0) - S[b][:, : a.size]).astype(np.float32)  # [N_D, K]
        t_idx, g_idx = _solve_assignment(cost.T)
        pair_cost = cost[g_idx, t_idx]
        w = conf[b, a][t_idx]
        total = total + np.float32(np.sum((w * pair_cost).astype(np.float32))) / np.float32(
            g_idx.shape[0]
        )
        valid += 1
    if valid == 0:
        return np.float32(0.0)
    return np.float32(total / np.float32(valid))
